# revision 1
# baseline (speedup 1.0000x reference)
"""GroupQueryAttention Bass kernel for Trainium2 (8 NeuronCores).

Problem: B=4, S=2048, E=1024, 16 Q-heads, 4 KV-heads (groups), head_dim=64.
Reference quirk: group g attends with K/V "head" g (of the 4 HPG slots), and the
output is flattened in (p, g, d) order: out channel = p*256 + g*64 + d.

Sharding: 8 cores = 4 batches x 2 sequence halves. Each core receives the full
x[b] (rows reordered so its own query half comes first -- attention is invariant
to key/value ordering) and computes a complete [1024, 1024] slice of the output.
No cross-core communication needed; the host concatenates slices.

Per-core dataflow (all fp32):
  1. PE-transpose x -> xT [e, s] (channels on partitions).
  2. QT = Wq^T x^T (+bq), KT (dup'd per group for row-packed QK), V_ext = x Wv
     augmented with a ones column per head (softmax denominator rides the PV
     matmul for free).
  3. Per head pair: scores^T[k, q] = K_g Q_h^T via row-packed (2 heads
     concurrent) matmuls; exp on ScalarE (scale=1/8 folded in; no max-subtract
     needed -- scores/8 ~ N(0,1), fp32-safe); PV accumulates over k-tiles with
     the ones column producing the denominator row.
  4. Reciprocal of all 16 denominator rows in one DVE op; per-head PE-broadcast
     of 1/denom and elementwise normalize; O-projection (+bo); DMA out.
"""

import numpy as np

import concourse.bass as bass
import concourse.tile as tile
from concourse import bacc, mybir
from concourse.bass_utils import run_bass_kernel_spmd
from concourse.masks import make_identity

B, S, E = 4, 2048, 1024
H, G, HPG, HD = 16, 4, 4, 64
KV = HPG * HD           # 256
SH = S // 2             # 1024 query rows per core
VX = HPG * (HD + 1)     # 260: V_ext row length (64 V cols + 1 ones col per head)
FP = mybir.dt.float32
AF = mybir.ActivationFunctionType
ALU = mybir.AluOpType
FPR = mybir.dt.float32r


def _r(ap):
    return ap.bitcast(FPR)

_CACHE = {}


def _body(tc, io):
    nc = tc.nc
    xb, Wq, Wk, Wv, Wo = io["xb"], io["Wq"], io["Wk"], io["Wv"], io["Wo"]
    bq, bk, bv, bo, out = io["bq"], io["bk"], io["bv"], io["bo"], io["out"]

    from contextlib import ExitStack

    with ExitStack() as es:
        const = es.enter_context(tc.tile_pool(name="const", bufs=1))
        ident = const.tile([128, 128], FP, tag="ident", name="ident")
        make_identity(nc, ident)
        ones = const.tile([1, 512], FP, tag="ones", name="ones")
        ones_st = const.tile([1, 512], FP, tag="ones_st", name="ones_st")
        nc.gpsimd.memset(ones_st, 1.0)
        nc.vector.tensor_copy(_r(ones), ones_st)
        bq_sb = const.tile([1, E], FP, tag="bq", name="bq")
        nc.sync.dma_start(_r(bq_sb), _r(bq))
        bk_sb = const.tile([1, KV], FP, tag="bk", name="bk")
        nc.sync.dma_start(_r(bk_sb), _r(bk))
        bo_sb = const.tile([1, E], FP, tag="bo", name="bo")
        nc.sync.dma_start(_r(bo_sb), _r(bo))
        # bv_ext: V bias per head + constant 1.0 in each head's ones slot.
        bvx = const.tile([1, VX], FP, tag="bvx", name="bvx")
        bvx_st = const.tile([1, VX], FP, tag="bvx_st", name="bvx_st")
        nc.gpsimd.memset(bvx_st, 1.0)
        for h in range(HPG):
            nc.sync.dma_start(bvx_st[0:1, h * 65 : h * 65 + 64], bv[0:1, h * 64 : (h + 1) * 64])
        nc.vector.tensor_copy(_r(bvx), bvx_st)
        denpack = const.tile([16, SH], FP, tag="denpack", name="denpack")
        recips = const.tile([16, SH], FP, tag="recips", name="recips")

        # Persist across projection + attention phases.
        pers = es.enter_context(tc.tile_pool(name="pers", bufs=1))
        qt_sb = [pers.tile([128, SH], FP, tag=f"qt{i}", name=f"qt{i}") for i in range(8)]
        kt_dup = [pers.tile([128, S], FP, tag=f"ktd{g}", name=f"ktd{g}") for g in range(G)]
        vx_sb = [pers.tile([128, VX], FP, tag=f"vx{st}", name=f"vx{st}") for st in range(16)]

        # ---------------- Phase A+B: transpose x, projections ----------------
        with tc.tile_pool(name="xtp", bufs=1) as xtp:
            xT = [xtp.tile([128, S], FP, tag=f"xT{e}", name=f"xT{e}") for e in range(8)]

            with (
                tc.tile_pool(name="xin", bufs=8) as xin,
                tc.tile_pool(name="trps", bufs=2, space="PSUM") as trps,
            ):
                for sg in range(4):
                    xts = []
                    for j in range(4):
                        t = xin.tile([128, E], FP, tag="xin", name="xin")
                        st = sg * 4 + j
                        nc.sync.dma_start(t, xb[st * 128 : (st + 1) * 128, :])
                        xts.append(t)
                    for et in range(8):
                        ps = trps.tile([128, 512], FP, tag="trp", name="trp")
                        for j in range(4):
                            nc.tensor.transpose(
                                ps[:, j * 128 : (j + 1) * 128],
                                xts[j][:, et * 128 : (et + 1) * 128],
                                ident,
                            )
                        nc.vector.tensor_copy(_r(xT[et][:, sg * 512 : (sg + 1) * 512]), ps)

            with (
                tc.tile_pool(name="wqs", bufs=16) as wqs,
                tc.tile_pool(name="wks", bufs=1) as wks,
                tc.tile_pool(name="wvxs", bufs=1) as wvxs,
                tc.tile_pool(name="pps", bufs=4, space="PSUM") as pps,
            ):
                wk_sb = []
                for et in range(8):
                    t = wks.tile([128, KV], FP, tag=f"wk{et}", name=f"wk{et}")
                    nc.sync.dma_start(_r(t), _r(Wk[et * 128 : (et + 1) * 128, :]))
                    wk_sb.append(t)
                wvx_sb = []
                for et in range(8):
                    tst = wvxs.tile([128, VX], FP, tag=f"wvxs{et}", name=f"wvxs{et}")
                    nc.gpsimd.memset(tst, 0.0)
                    for h in range(HPG):
                        nc.sync.dma_start(
                            tst[:, h * 65 : h * 65 + 64],
                            Wv[et * 128 : (et + 1) * 128, h * 64 : (h + 1) * 64],
                        )
                    t = wvxs.tile([128, VX], FP, tag=f"wvx{et}", name=f"wvx{et}")
                    nc.vector.tensor_copy(_r(t), tst)
                    wvx_sb.append(t)

                # QT[c, q] = Wq^T @ xT (+ bq outer ones)
                for ct in range(8):
                    wqt = []
                    for et in range(8):
                        w = wqs.tile([128, 128], FP, tag="wq", name="wq")
                        nc.sync.dma_start(
                            _r(w), _r(Wq[et * 128 : (et + 1) * 128, ct * 128 : (ct + 1) * 128])
                        )
                        wqt.append(w)
                    for qc in range(2):
                        ps = pps.tile([128, 512], FP, tag="pp", name="pp")
                        for et in range(8):
                            nc.tensor.matmul(
                                ps,
                                _r(wqt[et]),
                                _r(xT[et][:, qc * 512 : (qc + 1) * 512]),
                                start=(et == 0),
                                stop=False,
                            )
                        nc.tensor.matmul(
                            ps,
                            _r(bq_sb[0:1, ct * 128 : (ct + 1) * 128]),
                            _r(ones[0:1, 0:512]),
                            start=False,
                            stop=True,
                        )
                        nc.vector.tensor_copy(_r(qt_sb[ct][:, qc * 512 : (qc + 1) * 512]), ps)

                # KT[c, s] for all 2048 keys; store per-group duplicated across
                # both partition halves for row-packed QK^T.
                for ct in range(2):
                    g0, g1 = 2 * ct, 2 * ct + 1
                    for sc in range(4):
                        ps = pps.tile([128, 512], FP, tag="pp", name="pp")
                        for et in range(8):
                            nc.tensor.matmul(
                                ps,
                                _r(wk_sb[et][:, ct * 128 : (ct + 1) * 128]),
                                _r(xT[et][:, sc * 512 : (sc + 1) * 512]),
                                start=(et == 0),
                                stop=False,
                            )
                        nc.tensor.matmul(
                            ps,
                            _r(bk_sb[0:1, ct * 128 : (ct + 1) * 128]),
                            _r(ones[0:1, 0:512]),
                            start=False,
                            stop=True,
                        )
                        nc.vector.tensor_copy(
                            _r(kt_dup[g0][0:64, sc * 512 : (sc + 1) * 512]), ps[0:64, :]
                        )
                        nc.vector.tensor_copy(
                            _r(kt_dup[g1][64:128, sc * 512 : (sc + 1) * 512]), ps[64:128, :]
                        )
                    nc.sync.dma_start(_r(kt_dup[g0][64:128, :]), _r(kt_dup[g0][0:64, :]))
                    nc.sync.dma_start(_r(kt_dup[g1][0:64, :]), _r(kt_dup[g1][64:128, :]))

                # V_ext[s, (h, d|1)] = x @ Wv_ext (+ bv_ext outer ones)
                for st in range(16):
                    ps = pps.tile([128, VX], FP, tag="pp", name="pp")
                    for et in range(8):
                        nc.tensor.matmul(
                            ps,
                            _r(xT[et][:, st * 128 : (st + 1) * 128]),
                            _r(wvx_sb[et]),
                            start=(et == 0),
                            stop=False,
                        )
                    nc.tensor.matmul(
                        ps, _r(ones[0:1, 0:128]), _r(bvx), start=False, stop=True
                    )
                    nc.vector.tensor_copy(_r(vx_sb[st]), ps)

        # ---------------- Phase C: attention ----------------
        with (
            tc.tile_pool(name="aop", bufs=1) as aop,
            tc.tile_pool(name="wop", bufs=1) as wop,
        ):
            aoT = [aop.tile([128, SH], FP, tag=f"ao{t}", name=f"ao{t}") for t in range(8)]
            wo_sb = [wop.tile([128, E], FP, tag=f"wo{t}", name=f"wo{t}") for t in range(8)]
            for t in range(8):
                nc.sync.dma_start(_r(wo_sb[t]), _r(Wo[t * 128 : (t + 1) * 128, :]))

            with (
                tc.tile_pool(name="scp", bufs=2, space="PSUM") as scp,
                tc.tile_pool(name="pvp", bufs=2, space="PSUM") as pvp,
                tc.tile_pool(name="exps", bufs=4) as exps,
                tc.tile_pool(name="denp", bufs=2) as denp,
                tc.tile_pool(name="tmpp", bufs=2) as tmpp,
            ):
                for i in range(8):  # head pairs (2i, 2i+1), same group
                    g = i // 2
                    pvs = [pvp.tile([65, SH], FP, tag="pv", name="pv") for _ in range(2)]

                    def _pv(kt, exs, g=g, pvs=pvs):
                        for x2 in range(2):
                            for qc in range(2):
                                nc.tensor.matmul(
                                    pvs[x2][:, qc * 512 : (qc + 1) * 512],
                                    _r(vx_sb[kt][:, g * 65 : (g + 1) * 65]),
                                    _r(exs[x2][:, qc * 512 : (qc + 1) * 512]),
                                    start=(kt == 0),
                                    stop=(kt == 15),
                                )

                    prev = None  # (kt, [exA, exB]) one k-tile behind
                    for kt in range(16):
                        scs = [scp.tile([128, SH], FP, tag="sc", name="sc") for _ in range(2)]
                        for x2 in range(2):
                            for qc in range(2):
                                nc.tensor.matmul(
                                    scs[x2][:, qc * 512 : (qc + 1) * 512],
                                    _r(kt_dup[g][x2 * 64 : (x2 + 1) * 64, kt * 128 : (kt + 1) * 128]),
                                    _r(qt_sb[i][x2 * 64 : (x2 + 1) * 64, qc * 512 : (qc + 1) * 512]),
                                    start=True,
                                    stop=True,
                                    tile_position=(x2 * 64, 0),
                                )
                        exs = []
                        for x2 in range(2):
                            ex = exps.tile([128, SH], FP, tag="ex", name="ex")
                            nc.scalar.activation(_r(ex), scs[x2], AF.Exp, scale=0.125)
                            exs.append(ex)
                        if prev is not None:
                            _pv(*prev)
                        prev = (kt, exs)
                    _pv(*prev)
                    for x2 in range(2):
                        h = 2 * i + x2
                        hg, hp = h // 4, h % 4
                        tidx = 2 * hp + hg // 2
                        poff = (hg % 2) * 64
                        dn = denp.tile([65, SH], FP, tag="dn", name="dn")
                        nc.vector.tensor_copy(dn[64:65, :], pvs[x2][64:65, :])
                        nc.sync.dma_start(denpack[h : h + 1, :], dn[64:65, :])
                        if poff == 0:
                            nc.vector.tensor_copy(_r(aoT[tidx][0:64, :]), pvs[x2][0:64, :])
                        else:
                            tm = tmpp.tile([64, SH], FP, tag="tm", name="tm")
                            nc.vector.tensor_copy(_r(tm), pvs[x2][0:64, :])
                            nc.sync.dma_start(_r(aoT[tidx][64:128, :]), _r(tm))

            # ---------------- Phase D: normalize + O-projection ----------------
            nc.vector.reciprocal(recips, denpack)
            nc.vector.tensor_copy(_r(recips), recips)
            with (
                tc.tile_pool(name="stg", bufs=2) as stg,
                tc.tile_pool(name="bcp", bufs=2, space="PSUM") as bcp,
            ):
                for h in range(16):
                    hg, hp = h // 4, h % 4
                    tidx = 2 * hp + hg // 2
                    poff = (hg % 2) * 64
                    st = stg.tile([1, SH], FP, tag="st", name="st")
                    nc.sync.dma_start(_r(st), _r(recips[h : h + 1, :]))
                    bc = bcp.tile([128, SH], FP, tag="bc", name="bc")
                    for qc in range(2):
                        nc.tensor.matmul(
                            bc[:, qc * 512 : (qc + 1) * 512],
                            _r(ones[0:1, 0:128]),
                            _r(st[0:1, qc * 512 : (qc + 1) * 512]),
                            start=True,
                            stop=True,
                        )
                    nc.vector.tensor_tensor(
                        _r(aoT[tidx][poff : poff + 64, :]),
                        _r(aoT[tidx][poff : poff + 64, :]),
                        bc[poff : poff + 64, :],
                        ALU.mult,
                    )

            with (
                tc.tile_pool(name="ops", bufs=4, space="PSUM") as ops,
                tc.tile_pool(name="osb", bufs=2) as osb,
            ):
                for qt in range(8):
                    ot = osb.tile([128, E], FP, tag="ot", name="ot")
                    for oc in range(2):
                        ps = ops.tile([128, 512], FP, tag="op", name="op")
                        for ct in range(8):
                            nc.tensor.matmul(
                                ps,
                                _r(aoT[ct][:, qt * 128 : (qt + 1) * 128]),
                                _r(wo_sb[ct][:, oc * 512 : (oc + 1) * 512]),
                                start=(ct == 0),
                                stop=False,
                            )
                        nc.tensor.matmul(
                            ps,
                            _r(ones[0:1, 0:128]),
                            _r(bo_sb[0:1, oc * 512 : (oc + 1) * 512]),
                            start=False,
                            stop=True,
                        )
                        nc.vector.tensor_copy(ot[:, oc * 512 : (oc + 1) * 512], ps)
                    nc.sync.dma_start(out[qt * 128 : (qt + 1) * 128, :], ot)


def _build():
    if "nc" in _CACHE:
        return _CACHE["nc"]
    nc = bacc.Bacc(
        "TRN2", target_bir_lowering=False, debug=False, num_devices=8
    )
    io = {}
    io["xb"] = nc.dram_tensor("xb", [S, E], FP, kind="ExternalInput").ap()
    io["Wq"] = nc.dram_tensor("Wq", [E, E], FP, kind="ExternalInput").ap()
    io["Wk"] = nc.dram_tensor("Wk", [E, KV], FP, kind="ExternalInput").ap()
    io["Wv"] = nc.dram_tensor("Wv", [E, KV], FP, kind="ExternalInput").ap()
    io["Wo"] = nc.dram_tensor("Wo", [E, E], FP, kind="ExternalInput").ap()
    io["bq"] = nc.dram_tensor("bq", [1, E], FP, kind="ExternalInput").ap()
    io["bk"] = nc.dram_tensor("bk", [1, KV], FP, kind="ExternalInput").ap()
    io["bv"] = nc.dram_tensor("bv", [1, KV], FP, kind="ExternalInput").ap()
    io["bo"] = nc.dram_tensor("bo", [1, E], FP, kind="ExternalInput").ap()
    io["out"] = nc.dram_tensor("out", [SH, E], FP, kind="ExternalOutput").ap()
    with tile.TileContext(nc) as tc:
        _body(tc, io)
    nc.compile()
    _CACHE["nc"] = nc
    return nc


def _run(inputs, trace=False):
    x = np.ascontiguousarray(np.asarray(inputs["x"], dtype=np.float32))
    w = {k: np.ascontiguousarray(np.asarray(inputs[k], dtype=np.float32)) for k in
         ("Wq", "Wk", "Wv", "Wo")}
    bias = {k: np.ascontiguousarray(
        np.asarray(inputs[k], dtype=np.float32).reshape(1, -1)) for k in
        ("bq", "bk", "bv", "bo")}

    nc = _build()
    in_maps = []
    for b in range(B):
        for hf in range(2):
            if hf == 0:
                xbv = x[b]
            else:
                xbv = np.ascontiguousarray(
                    np.concatenate([x[b, SH:], x[b, :SH]], axis=0)
                )
            m = {"xb": xbv}
            m.update(w)
            m.update(bias)
            in_maps.append(m)

    res = run_bass_kernel_spmd(nc, in_maps, list(range(8)), trace=trace)
    out = np.empty((B, S, E), dtype=np.float32)
    for b in range(B):
        for hf in range(2):
            out[b, hf * SH : (hf + 1) * SH] = res.results[b * 2 + hf]["out"]
    return out, res


def kernel(**inputs):
    out, _ = _run(inputs, trace=False)
    return out



# revision 5
# speedup vs baseline: 1.3904x; 1.3904x over previous
"""GroupQueryAttention Bass kernel for Trainium2 (8 NeuronCores).

Problem: B=4, S=2048, E=1024, 16 Q-heads, 4 KV-heads (groups), head_dim=64.
Reference quirk: group g attends with K/V "head" g (of the 4 HPG slots), and the
output is flattened in (p, g, d) order: out channel = p*256 + g*64 + d.

Sharding: 8 cores = 4 batches x 2 sequence halves. Each core receives the full
x[b] (rows reordered so its own query half comes first -- attention is invariant
to key/value ordering) and computes a complete [1024, 1024] slice of the output.
No cross-core communication needed; the host concatenates slices.

Per-core dataflow (all fp32):
  1. PE-transpose x -> xT [e, s] (channels on partitions).
  2. QT = Wq^T x^T (+bq), KT (dup'd per group for row-packed QK), V_ext = x Wv
     augmented with a ones column per head (softmax denominator rides the PV
     matmul for free).
  3. Attention is a ScalarE-exp-bound pipeline (~1.1us per [128,1024] tile, 256
     tiles/core). Per head pair, score tiles rotate through a 2-buffer PSUM pool
     with one tile per (k-tile, head) half-step, so exp on ScalarE streams
     back-to-back while the PE runs QK one half-step ahead and PV one k-tile
     behind. exp folds in scale=1/8; no max-subtract needed (scores/8 ~ N(0,1),
     fp32-safe).
  4. Per-pair epilogue on otherwise-idle engines (DVE/GpSimd/DMA): copy
     unnormalized PV + denominator out of PSUM (releasing the accumulator banks
     for the next pair), reciprocal, GpSimd partition-broadcast of 1/den, and an
     in-place aligned normalize. O-projection (+bo) at the end.
"""

import numpy as np

import concourse.bass as bass
import concourse.tile as tile
from concourse import bacc, mybir
from concourse.bass_utils import run_bass_kernel_spmd
from concourse.masks import make_identity

B, S, E = 4, 2048, 1024
H, G, HPG, HD = 16, 4, 4, 64
KV = HPG * HD           # 256
SH = S // 2             # 1024 query rows per core
VX = HPG * (HD + 1)     # 260: V_ext row length (64 V cols + 1 ones col per head)
FP = mybir.dt.float32
AF = mybir.ActivationFunctionType
ALU = mybir.AluOpType
FPR = mybir.dt.float32r


def _r(ap):
    return ap.bitcast(FPR)

_CACHE = {}


def _body(tc, io):
    nc = tc.nc
    xb, Wq, Wk, Wv, Wo = io["xb"], io["Wq"], io["Wk"], io["Wv"], io["Wo"]
    bq, bk, bv, bo, out = io["bq"], io["bk"], io["bv"], io["bo"], io["out"]

    from contextlib import ExitStack

    with ExitStack() as es:
        const = es.enter_context(tc.tile_pool(name="const", bufs=1))
        ident = const.tile([128, 128], FP, tag="ident", name="ident")
        make_identity(nc, ident)
        ones = const.tile([1, 512], FP, tag="ones", name="ones")
        ones_st = const.tile([1, 512], FP, tag="ones_st", name="ones_st")
        nc.gpsimd.memset(ones_st, 1.0)
        nc.vector.tensor_copy(_r(ones), ones_st)
        bq_sb = const.tile([1, E], FP, tag="bq", name="bq")
        nc.sync.dma_start(_r(bq_sb), _r(bq))
        bk_sb = const.tile([1, KV], FP, tag="bk", name="bk")
        nc.sync.dma_start(_r(bk_sb), _r(bk))
        bo_sb = const.tile([1, E], FP, tag="bo", name="bo")
        nc.sync.dma_start(_r(bo_sb), _r(bo))
        # bv_ext: V bias per head + constant 1.0 in each head's ones slot.
        bvx = const.tile([1, VX], FP, tag="bvx", name="bvx")
        bvx_st = const.tile([1, VX], FP, tag="bvx_st", name="bvx_st")
        nc.gpsimd.memset(bvx_st, 1.0)
        for h in range(HPG):
            nc.sync.dma_start(bvx_st[0:1, h * 65 : h * 65 + 64], bv[0:1, h * 64 : (h + 1) * 64])
        nc.vector.tensor_copy(_r(bvx), bvx_st)

        # Persist across projection + attention phases.
        pers = es.enter_context(tc.tile_pool(name="pers", bufs=1))
        qt_sb = [pers.tile([128, SH], FP, tag=f"qt{i}", name=f"qt{i}") for i in range(8)]
        kt_dup = [pers.tile([128, S], FP, tag=f"ktd{g}", name=f"ktd{g}") for g in range(G)]
        vx_sb = [pers.tile([128, VX], FP, tag=f"vx{st}", name=f"vx{st}") for st in range(16)]

        # ---------------- Phase A+B: transpose x, projections ----------------
        with tc.tile_pool(name="xtp", bufs=1) as xtp:
            xT = [xtp.tile([128, S], FP, tag=f"xT{e}", name=f"xT{e}") for e in range(8)]

            with (
                tc.tile_pool(name="xin", bufs=8) as xin,
                tc.tile_pool(name="trps", bufs=2, space="PSUM") as trps,
            ):
                for sg in range(4):
                    xts = []
                    for j in range(4):
                        t = xin.tile([128, E], FP, tag="xin", name="xin")
                        st = sg * 4 + j
                        nc.sync.dma_start(t, xb[st * 128 : (st + 1) * 128, :])
                        xts.append(t)
                    for et in range(8):
                        ps = trps.tile([128, 512], FP, tag="trp", name="trp")
                        for j in range(4):
                            nc.tensor.transpose(
                                ps[:, j * 128 : (j + 1) * 128],
                                xts[j][:, et * 128 : (et + 1) * 128],
                                ident,
                            )
                        nc.vector.tensor_copy(_r(xT[et][:, sg * 512 : (sg + 1) * 512]), ps)

            with (
                tc.tile_pool(name="wqs", bufs=16) as wqs,
                tc.tile_pool(name="wks", bufs=1) as wks,
                tc.tile_pool(name="wvxs", bufs=1) as wvxs,
                tc.tile_pool(name="pps", bufs=4, space="PSUM") as pps,
            ):
                wk_sb = []
                for et in range(8):
                    t = wks.tile([128, KV], FP, tag=f"wk{et}", name=f"wk{et}")
                    nc.sync.dma_start(_r(t), _r(Wk[et * 128 : (et + 1) * 128, :]))
                    wk_sb.append(t)
                wvx_sb = []
                for et in range(8):
                    tst = wvxs.tile([128, VX], FP, tag=f"wvxs{et}", name=f"wvxs{et}")
                    nc.gpsimd.memset(tst, 0.0)
                    for h in range(HPG):
                        nc.sync.dma_start(
                            tst[:, h * 65 : h * 65 + 64],
                            Wv[et * 128 : (et + 1) * 128, h * 64 : (h + 1) * 64],
                        )
                    t = wvxs.tile([128, VX], FP, tag=f"wvx{et}", name=f"wvx{et}")
                    nc.vector.tensor_copy(_r(t), tst)
                    wvx_sb.append(t)

                # QT[c, q] = Wq^T @ xT (+ bq outer ones)
                for ct in range(8):
                    wqt = []
                    for et in range(8):
                        w = wqs.tile([128, 128], FP, tag="wq", name="wq")
                        nc.sync.dma_start(
                            _r(w), _r(Wq[et * 128 : (et + 1) * 128, ct * 128 : (ct + 1) * 128])
                        )
                        wqt.append(w)
                    for qc in range(2):
                        ps = pps.tile([128, 512], FP, tag="pp", name="pp")
                        for et in range(8):
                            nc.tensor.matmul(
                                ps,
                                _r(wqt[et]),
                                _r(xT[et][:, qc * 512 : (qc + 1) * 512]),
                                start=(et == 0),
                                stop=False,
                            )
                        nc.tensor.matmul(
                            ps,
                            _r(bq_sb[0:1, ct * 128 : (ct + 1) * 128]),
                            _r(ones[0:1, 0:512]),
                            start=False,
                            stop=True,
                        )
                        nc.vector.tensor_copy(_r(qt_sb[ct][:, qc * 512 : (qc + 1) * 512]), ps)

                # KT[c, s] for all 2048 keys; store per-group duplicated across
                # both partition halves for row-packed QK^T.
                for ct in range(2):
                    g0, g1 = 2 * ct, 2 * ct + 1
                    for sc in range(4):
                        ps = pps.tile([128, 512], FP, tag="pp", name="pp")
                        for et in range(8):
                            nc.tensor.matmul(
                                ps,
                                _r(wk_sb[et][:, ct * 128 : (ct + 1) * 128]),
                                _r(xT[et][:, sc * 512 : (sc + 1) * 512]),
                                start=(et == 0),
                                stop=False,
                            )
                        nc.tensor.matmul(
                            ps,
                            _r(bk_sb[0:1, ct * 128 : (ct + 1) * 128]),
                            _r(ones[0:1, 0:512]),
                            start=False,
                            stop=True,
                        )
                        nc.vector.tensor_copy(
                            _r(kt_dup[g0][0:64, sc * 512 : (sc + 1) * 512]), ps[0:64, :]
                        )
                        nc.vector.tensor_copy(
                            _r(kt_dup[g1][64:128, sc * 512 : (sc + 1) * 512]), ps[64:128, :]
                        )
                    nc.sync.dma_start(_r(kt_dup[g0][64:128, :]), _r(kt_dup[g0][0:64, :]))
                    nc.sync.dma_start(_r(kt_dup[g1][0:64, :]), _r(kt_dup[g1][64:128, :]))

                # V_ext[s, (h, d|1)] = x @ Wv_ext (+ bv_ext outer ones)
                for st in range(16):
                    ps = pps.tile([128, VX], FP, tag="pp", name="pp")
                    for et in range(8):
                        nc.tensor.matmul(
                            ps,
                            _r(xT[et][:, st * 128 : (st + 1) * 128]),
                            _r(wvx_sb[et]),
                            start=(et == 0),
                            stop=False,
                        )
                    nc.tensor.matmul(
                        ps, _r(ones[0:1, 0:128]), _r(bvx), start=False, stop=True
                    )
                    nc.vector.tensor_copy(_r(vx_sb[st]), ps)

        # ---------------- Phase C: attention ----------------
        # ScalarE exp is the bottleneck engine; the PSUM budget makes it
        # streamable: 2 rotating score tiles (2 banks each) + 2 persistent PV
        # accumulators (2 banks each) = 8 banks. One score tile is allocated
        # per (kt, head) half-step, so consecutive half-steps alternate
        # buffers and QK(kt+1) only waits on exp(kt) of the same parity.
        with (
            tc.tile_pool(name="aop", bufs=1) as aop,
            tc.tile_pool(name="wop", bufs=1) as wop,
        ):
            aoT = [aop.tile([128, SH], FP, tag=f"ao{t}", name=f"ao{t}") for t in range(8)]
            wo_sb = [wop.tile([128, E], FP, tag=f"wo{t}", name=f"wo{t}") for t in range(8)]
            for t in range(8):
                nc.sync.dma_start(_r(wo_sb[t]), _r(Wo[t * 128 : (t + 1) * 128, :]))

            with (
                tc.tile_pool(name="scp", bufs=2, space="PSUM") as scp,
                tc.tile_pool(name="pvp", bufs=2, space="PSUM") as pvp,
                tc.tile_pool(name="exps", bufs=5) as exps,
                tc.tile_pool(name="denp", bufs=1) as denp,
                tc.tile_pool(name="tmpp", bufs=1) as tmpp,
                tc.tile_pool(name="rbp", bufs=1) as rbp,
            ):
                for i in range(8):  # head pairs (2i, 2i+1), same group
                    g = i // 2
                    pvs = [pvp.tile([65, SH], FP, tag="pv", name="pv") for _ in range(2)]

                    def _pv(kt, exs, g=g, pvs=pvs):
                        for x2 in range(2):
                            for qc in range(2):
                                nc.tensor.matmul(
                                    pvs[x2][:, qc * 512 : (qc + 1) * 512],
                                    _r(vx_sb[kt][:, g * 65 : (g + 1) * 65]),
                                    _r(exs[x2][:, qc * 512 : (qc + 1) * 512]),
                                    start=(kt == 0),
                                    stop=(kt == 15),
                                )

                    prev = None  # (kt, [exA, exB]) one k-tile behind
                    for kt in range(16):
                        scs = [scp.tile([128, SH], FP, tag="sc", name="sc") for _ in range(2)]
                        for x2 in range(2):
                            for qc in range(2):
                                nc.tensor.matmul(
                                    scs[x2][:, qc * 512 : (qc + 1) * 512],
                                    _r(kt_dup[g][x2 * 64 : (x2 + 1) * 64, kt * 128 : (kt + 1) * 128]),
                                    _r(qt_sb[i][x2 * 64 : (x2 + 1) * 64, qc * 512 : (qc + 1) * 512]),
                                    start=True,
                                    stop=True,
                                    tile_position=(x2 * 64, 0),
                                )
                        exs = []
                        for x2 in range(2):
                            ex = exps.tile([128, SH], FP, tag="ex", name="ex")
                            nc.scalar.activation(_r(ex), scs[x2], AF.Exp, scale=0.125)
                            exs.append(ex)
                        if prev is not None:
                            _pv(*prev)
                        prev = (kt, exs)
                    _pv(*prev)

                    # Per-pair epilogue on DVE/GpSimd/DMA, hidden under the
                    # next pairs' exp stream. Both heads of a pair share poff.
                    h0 = 2 * i
                    hg, hp = h0 // 4, h0 % 4
                    poff = (hg % 2) * 64

                    # Denominators: aligned copy out of PSUM row 64, DMA each
                    # row into partition 0/1 of one tile, one reciprocal.
                    dn = denp.tile([65, SH], FP, tag="dn", name="dn")
                    dnp = denp.tile([2, SH], FP, tag="dnp", name="dnp")
                    for x2 in range(2):
                        nc.vector.tensor_copy(dn[64:65, :], pvs[x2][64:65, :])
                        nc.sync.dma_start(_r(dnp[x2 : x2 + 1, :]), _r(dn[64:65, :]))
                    rc = denp.tile([2, SH], FP, tag="rc", name="rc")
                    nc.vector.reciprocal(rc, dnp)
                    rc1 = denp.tile([1, SH], FP, tag="rc1", name="rc1")
                    nc.sync.dma_start(_r(rc1), _r(rc[1:2, :]))

                    # Unnormalized PV out of PSUM (frees the accumulators).
                    for x2 in range(2):
                        h = 2 * i + x2
                        tidx = 2 * (h % 4) + (h // 4) // 2
                        if poff == 0:
                            nc.vector.tensor_copy(
                                _r(aoT[tidx][0:64, :]), pvs[x2][0:64, :]
                            )
                        else:
                            tm = tmpp.tile([64, SH], FP, tag="tm", name="tm")
                            nc.vector.tensor_copy(_r(tm), pvs[x2][0:64, :])
                            nc.sync.dma_start(_r(aoT[tidx][64:128, :]), _r(tm))

                    # Broadcast 1/den across all 128 partitions (GpSimd), then
                    # normalize each head's aoT half in place, fully aligned.
                    for x2, rsrc in ((0, rc[0:1, :]), (1, rc1[0:1, :])):
                        h = 2 * i + x2
                        tidx = 2 * (h % 4) + (h // 4) // 2
                        rb = rbp.tile([128, SH], FP, tag="rb", name="rb")
                        nc.gpsimd.partition_broadcast(rb, rsrc)
                        nc.vector.tensor_tensor(
                            _r(aoT[tidx][poff : poff + 64, :]),
                            _r(aoT[tidx][poff : poff + 64, :]),
                            _r(rb[poff : poff + 64, :]),
                            ALU.mult,
                        )

            # ---------------- Phase D: O-projection ----------------
            with (
                tc.tile_pool(name="ops", bufs=4, space="PSUM") as ops,
                tc.tile_pool(name="osb", bufs=2) as osb,
            ):
                for qt in range(8):
                    ot = osb.tile([128, E], FP, tag="ot", name="ot")
                    for oc in range(2):
                        ps = ops.tile([128, 512], FP, tag="op", name="op")
                        for ct in range(8):
                            nc.tensor.matmul(
                                ps,
                                _r(aoT[ct][:, qt * 128 : (qt + 1) * 128]),
                                _r(wo_sb[ct][:, oc * 512 : (oc + 1) * 512]),
                                start=(ct == 0),
                                stop=False,
                            )
                        nc.tensor.matmul(
                            ps,
                            _r(ones[0:1, 0:128]),
                            _r(bo_sb[0:1, oc * 512 : (oc + 1) * 512]),
                            start=False,
                            stop=True,
                        )
                        nc.vector.tensor_copy(ot[:, oc * 512 : (oc + 1) * 512], ps)
                    nc.sync.dma_start(out[qt * 128 : (qt + 1) * 128, :], ot)


def _build():
    if "nc" in _CACHE:
        return _CACHE["nc"]
    nc = bacc.Bacc(
        "TRN2", target_bir_lowering=False, debug=False, num_devices=8
    )
    io = {}
    io["xb"] = nc.dram_tensor("xb", [S, E], FP, kind="ExternalInput").ap()
    io["Wq"] = nc.dram_tensor("Wq", [E, E], FP, kind="ExternalInput").ap()
    io["Wk"] = nc.dram_tensor("Wk", [E, KV], FP, kind="ExternalInput").ap()
    io["Wv"] = nc.dram_tensor("Wv", [E, KV], FP, kind="ExternalInput").ap()
    io["Wo"] = nc.dram_tensor("Wo", [E, E], FP, kind="ExternalInput").ap()
    io["bq"] = nc.dram_tensor("bq", [1, E], FP, kind="ExternalInput").ap()
    io["bk"] = nc.dram_tensor("bk", [1, KV], FP, kind="ExternalInput").ap()
    io["bv"] = nc.dram_tensor("bv", [1, KV], FP, kind="ExternalInput").ap()
    io["bo"] = nc.dram_tensor("bo", [1, E], FP, kind="ExternalInput").ap()
    io["out"] = nc.dram_tensor("out", [SH, E], FP, kind="ExternalOutput").ap()
    with tile.TileContext(nc) as tc:
        _body(tc, io)
    nc.compile()
    _CACHE["nc"] = nc
    return nc


def _run(inputs, trace=False):
    x = np.ascontiguousarray(np.asarray(inputs["x"], dtype=np.float32))
    w = {k: np.ascontiguousarray(np.asarray(inputs[k], dtype=np.float32)) for k in
         ("Wq", "Wk", "Wv", "Wo")}
    bias = {k: np.ascontiguousarray(
        np.asarray(inputs[k], dtype=np.float32).reshape(1, -1)) for k in
        ("bq", "bk", "bv", "bo")}

    nc = _build()
    in_maps = []
    for b in range(B):
        for hf in range(2):
            if hf == 0:
                xbv = x[b]
            else:
                xbv = np.ascontiguousarray(
                    np.concatenate([x[b, SH:], x[b, :SH]], axis=0)
                )
            m = {"xb": xbv}
            m.update(w)
            m.update(bias)
            in_maps.append(m)

    res = run_bass_kernel_spmd(nc, in_maps, list(range(8)), trace=trace)
    out = np.empty((B, S, E), dtype=np.float32)
    for b in range(B):
        for hf in range(2):
            out[b, hf * SH : (hf + 1) * SH] = res.results[b * 2 + hf]["out"]
    return out, res


def kernel(**inputs):
    out, _ = _run(inputs, trace=False)
    return out
